# revision 19
# baseline (speedup 1.0000x reference)
"""Trainium2 Bass kernel for nn_Decoder_TRANSFORMER_14791867367496.

The reference decoder is affine in the positions: each frame step is
    pos_{t+1} = pos_t @ M + (d_t[b] + g[b,j]),   M = I + W_pe @ W3  (3x3)
(with W_final = [W1; W2; W3] split along its 768 input rows), so the whole
60-step scan has a closed form

    out[b, j, :, t] = X[b, j, :] @ Q_t + r_t[b, :]

where X = initial_grid,
    Q_t = M^t + (W_pe @ W2) @ S_t,          S_t = sum_{k<t} M^k
    r_t[b] = h @ S_t + D_t[b],              D_t = sum_{s=1..t} d_s M^{t-s}
    d_t[b] = (emb_table[t] + z @ W_clip + b_clip) @ W1
    h      = b_pe @ (W2 + W3) + b_final

All of Q/r are tiny (3x3 / per-batch 3-vectors) and are computed on the host
in float64.  The device kernel is then a single affine map per point
([3 feats + bias] -> 180 outputs) and is purely output-bandwidth bound
(94 MB of f32 output; measured per-core DMA saturation ~422 GB/s).

Precision trick: fp32 operands are split into two bf16 chunks
(x = x0+x1, 8 mantissa bits each) and the cross terms with a+b <= 1 are
summed IN A SINGLE MATMUL by stacking them along the contraction dim:
rows [x0 x0 x1] paired against [q0 q1 q0] per feature, plus two ones-rows
paired against the two bf16 chunks of the per-batch bias r.  bf16 products
are exact in fp32; dropped second-order terms are ~2^-18 (measured rel err
2.4e-6 vs the 2e-4 gate).  Per point-pair-tile the K-stack is 11 rows x 2
tiles = K=22, N=2*180=360 (block-diagonal rhs).

Sharding: data-parallel over batch - each of the 8 cores handles 4 batches
(16384 points = 128 point-tiles = 64 packed matmuls).  Output streams out
in fully-linear ~1.47 MB DMAs, the first group split 1/1/2/2/2 matmuls so
the stream starts right after matmul 0's copy.

Ramp/pipeline design (steady state is DMA-saturated, so exec time =
stream time + ramp latency; DMA completion semaphores lag the trigger's
descriptor-gen by a fixed ~1.4 us, descriptor-gen blocks the issuing
engine ~0.6-1.0 us per DMA, and all SBUF operands live in ONE sbuf tensor
so multi-part input loads stay fully linear):
  - SP ring carries ONE tiny head DMA (matmul 0's stationary + rhs
    columns) and then only the output DMAs.
  - ACT ring carries the bulk input as three linear DMAs (rest of group
    0; group 1; groups 2-7 + remaining rhs), each with its own completion
    semaphore (a DMA's +16 arrives as 16 per-engine +1 increments that
    interleave across DMAs, so cumulative thresholds on a shared
    semaphore would be racy).  The ACT engine is free again by the time
    its first PSUM copy is due.
  - Six stage buffers so the PSUM->SBUF copies never wait on the ~1.4 us
    out-DMA completion latency chain.
PE runs the matmuls, DVE/ACT alternate PSUM->SBUF copies, SP streams the
output.  The device program is raw Bacc with hand-rolled per-edge
semaphores.
"""

import numpy as np

BS, NFRAMES, NJOINTS, NFEATS, LATENT, CLIP = 32, 60, 4096, 3, 256, 512
NCORES = 8
B_PER_CORE = BS // NCORES                  # 4
PTS = B_PER_CORE * NJOINTS                 # 16384 points per core
NTILES = PTS // 128                        # 128 point-tiles per core
GROUPS = 8                                 # output DMA groups
TPG = NTILES // GROUPS                     # 16 tiles per group
FC = NFEATS * NFRAMES                      # 180 output columns per point
KR = 11                                    # K-stack rows per tile (3*3 + 2 bias)
PAIR = 2                                   # tiles fused per matmul
MM_PER_G = TPG // PAIR                     # 8 matmuls per group
NMM = GROUPS * MM_PER_G                    # 64 matmuls per core
XCH = [0, 0, 1]                            # x-chunk index per K row (per feat)
QCH = [0, 1, 0]                            # q-chunk index per K row (per feat)
K2 = PAIR * KR                             # matmul contraction dim (22)
XT_COLS = NMM * 128                        # 8192 stationary columns
RHS_COLS = B_PER_CORE * PAIR * FC          # 1440 moving columns
HEAD_X = 2 * 128                           # stationary cols in the head DMA
HEAD = HEAD_X + PAIR * FC                  # head DMA cols (616)
W_COLS = XT_COLS + RHS_COLS                # single sbuf tensor width (9632)
# group-0 output split in point-tiles per DMA; matmul 0 is itself split
# into two single-tile matmuls (half-N) with their own half-copies so the
# first out-DMA fires as early as possible.
G0_SPLIT_W = ((1, 1, 0), (2, 0, 1), (4, 0, 2), (8, 2, 3), (16, 4, 5))
# (w1, cpv_need, cpa_need): DMA covers tiles [prev_w1, w1)


def _xt_col(c):
    """wsb column of stationary (xt) column c."""
    return c if c < HEAD_X else PAIR * FC + c


def _rhs_col(r):
    """wsb column of moving (rhs) column r."""
    return HEAD_X + r if r < PAIR * FC else XT_COLS + r


def _split2(a):
    """Split f32 array into two bf16 chunks whose sum reproduces ~16
    mantissa bits.  Returned as f32 arrays holding bf16-representable
    values."""
    import ml_dtypes
    bf = ml_dtypes.bfloat16
    a = np.asarray(a, np.float32)
    a0 = a.astype(bf).astype(np.float32)
    a1 = (a - a0).astype(bf).astype(np.float32)
    return a0, a1


def _precompute(z, W_pe, b_pe, W_clip, b_clip, emb_table, W_final, b_final):
    """Host-side f64 computation of the closed-form coefficients.

    Returns Q_all [3, 180] and r_all [32, 180], column layout c = f*60 + t
    (matching the [.., 3, 60] innermost layout of the output)."""
    f64 = np.float64
    W_pe64 = np.asarray(W_pe, f64)
    W_fin = np.asarray(W_final, f64)
    W1, W2, W3 = W_fin[:LATENT], W_fin[LATENT:2 * LATENT], W_fin[2 * LATENT:]
    M = np.eye(3) + W_pe64 @ W3
    Gm = W_pe64 @ W2
    b_pe64 = np.asarray(b_pe, f64)
    h = b_pe64 @ W2 + b_pe64 @ W3 + np.asarray(b_final, f64)
    z_proj = np.asarray(z, f64) @ np.asarray(W_clip, f64) + np.asarray(b_clip, f64)
    d = (np.asarray(emb_table, f64)[None, :, :] + z_proj[:, None, :]) @ W1  # [32,60,3]

    Q = np.zeros((NFRAMES, 3, 3))
    R = np.zeros((NFRAMES, BS, 3))
    Q[0] = np.eye(3)
    Mt = np.eye(3)
    S = np.zeros((3, 3))
    D = np.zeros((BS, 3))
    for t in range(1, NFRAMES):
        S = S + Mt
        Mt = Mt @ M
        D = D @ M + d[:, t, :]
        Q[t] = Mt + Gm @ S
        R[t] = h @ S + D
    Q_all = Q.transpose(1, 2, 0).reshape(3, FC)     # [k, f*60+t]
    r_all = R.transpose(1, 2, 0).reshape(BS, FC)    # [b, f*60+t]
    return Q_all.astype(np.float32), r_all.astype(np.float32)


N_PS = 8      # psum slots (one bank each; a group cycles all 8)
N_STAGE = 6   # stage buffers


def _copy_seq(j):
    """(engine, 1-based position of copy j within that engine's stream).

    Copies alternate DVE/ACT by matmul index so both engines share every
    group's copy wall.  Matmul 0's copy is split into two halves (DVE pos
    1 / ACT pos 1), which shifts the ACT positions by one."""
    if j % 2 == 0:
        return "v", j // 2 + 1
    return "a", (j + 1) // 2 + 1


def _build_bass():
    import concourse.mybir as mybir
    from concourse import bacc
    from concourse.bass import ts

    f32 = mybir.dt.float32
    bf16 = mybir.dt.bfloat16
    nc = bacc.Bacc(None, target_bir_lowering=False)
    # bulk boundaries in xt-column space: rest of group 0 | group 1 |
    # groups 2-7 (the last bulk also carries the remaining rhs columns)
    b1_cols = MM_PER_G * 128 - HEAD_X
    b2_cols = MM_PER_G * 128
    b3_cols = (GROUPS - 2) * MM_PER_G * 128 + (B_PER_CORE - 1) * PAIR * FC
    head = nc.dram_tensor("head", [K2, HEAD], bf16, kind="ExternalInput")
    bulk1 = nc.dram_tensor("bulk1", [K2, b1_cols], bf16, kind="ExternalInput")
    bulk2 = nc.dram_tensor("bulk2", [K2, b2_cols], bf16, kind="ExternalInput")
    bulk3 = nc.dram_tensor("bulk3", [K2, b3_cols], bf16, kind="ExternalInput")
    out = nc.dram_tensor("out", [PTS, FC], f32, kind="ExternalOutput")
    out_v = out[:].rearrange("(g j w) c -> g j (w c)", g=GROUPS, j=128, w=TPG)

    from contextlib import ExitStack
    ctx = ExitStack()
    wsb = ctx.enter_context(nc.sbuf_tensor("wsb", [K2, W_COLS], bf16))
    stage = [ctx.enter_context(
        nc.sbuf_tensor(f"stage{i}", [128, TPG * FC], f32))
        for i in range(N_STAGE)]
    psum = [ctx.enter_context(
        nc.psum_tensor(f"psum{i}", [128, PAIR * FC], f32))
        for i in range(N_PS)]
    s_head = ctx.enter_context(nc.semaphore("s_head"))
    # one semaphore per bulk DMA: a DMA's +16 arrives as 16 per-engine +1
    # increments which interleave across DMAs on the same queue, so
    # cumulative thresholds on a shared semaphore would be racy
    s_bulk = [ctx.enter_context(nc.semaphore(f"s_bulk{i}")) for i in range(3)]
    s_pe = ctx.enter_context(nc.semaphore("s_pe"))
    s_cpv = ctx.enter_context(nc.semaphore("s_cpv"))
    s_cpa = ctx.enter_context(nc.semaphore("s_cpa"))
    s_slot = [ctx.enter_context(nc.semaphore(f"s_slot{i}"))
              for i in range(N_STAGE)]

    # ---- input DMAs ----
    # SP ring: one tiny head DMA (matmul 0+1 stationary cols + local batch
    # 0's rhs), then nothing but output.
    nc.sync.dma_start(out=wsb[:, :HEAD], in_=head[:]).then_inc(s_head, 16)
    # ACT ring: bulk input
    c0 = HEAD
    for i, (t, w) in enumerate(((bulk1, b1_cols), (bulk2, b2_cols),
                                (bulk3, b3_cols))):
        nc.scalar.dma_start(out=wsb[:, c0:c0 + w],
                            in_=t[:]).then_inc(s_bulk[i], 16)
        c0 += w

    def copies(engine, s_cp_self, g, parity):
        st = stage[g % N_STAGE]
        for sp in range(parity, MM_PER_G, 2):
            j = g * MM_PER_G + sp
            if sp == parity and g >= N_STAGE:
                # stage slot reuse: wait for every out-DMA that read it
                engine.wait_ge(s_slot[g % N_STAGE],
                               16 * slot_reads_before[g])
            if g == 0 and sp == parity:
                # group 0 starts with the two half-copies of matmul 0
                # (psum banks 0 and 1 hold the two half-N results)
                engine.wait_ge(s_pe, parity + 1)
                half = psum[parity][:, :FC]
                dst = st[:, parity * FC:(parity + 1) * FC]
                if parity == 0:
                    nc.vector.tensor_copy(out=dst, in_=half).then_inc(
                        s_cp_self, 1)
                    continue
                nc.scalar.copy(out=dst, in_=half).then_inc(s_cp_self, 1)
            engine.wait_ge(s_pe, j + 2)   # +1 for the extra half matmul
            if parity == 0:
                nc.vector.tensor_copy(
                    out=st[:, ts(sp, PAIR * FC)],
                    in_=psum[(j + 1) % N_PS][:],
                ).then_inc(s_cp_self, 1)
            else:
                nc.scalar.copy(
                    out=st[:, ts(sp, PAIR * FC)],
                    in_=psum[(j + 1) % N_PS][:],
                ).then_inc(s_cp_self, 1)

    # number of completed out-DMA incs required on slot g%N_STAGE before
    # group g may overwrite it (group 0's stage goes out as split DMAs)
    dma_count = {0: len(G0_SPLIT_W)}
    slot_reads_before = {}
    seen = [0] * N_STAGE
    for g in range(GROUPS):
        slot_reads_before[g] = seen[g % N_STAGE]
        seen[g % N_STAGE] += dma_count.get(g, 1)

    for g in range(GROUPS):
        copies(nc.scalar, s_cpa, g, 1)

    # ---- DVE: even-slot copies ----
    for g in range(GROUPS):
        copies(nc.vector, s_cpv, g, 0)

    # ---- PE: matmuls ----
    # matmul 0 runs as two single-tile (half-N) matmuls so its first half
    # copy (and the first out-DMA) fires half a matmul earlier
    for j in range(NMM):
        g, sp = j // MM_PER_G, j % MM_PER_G
        lb = g // 2
        if j == 0:
            nc.tensor.wait_ge(s_head, 16)
            # half-N matmuls with the full K=22 stationary: the rhs is
            # block-diagonal, so the off-tile stationary rows multiply
            # zero columns and the halves stay exact.  Each half gets its
            # own psum bank (PE writes must start at a bank base), which
            # shifts all later slot assignments by one.
            r0 = _rhs_col(0)
            for a in range(PAIR):
                nc.tensor.matmul(
                    psum[a][:, :FC],
                    wsb[:, 0:128],
                    wsb[:, r0 + a * FC:r0 + (a + 1) * FC],
                    start=True, stop=True,
                ).then_inc(s_pe, 1)
            continue
        if j == 2:
            nc.tensor.wait_ge(s_bulk[0], 16)      # bulk1: rest of group 0
        elif j == MM_PER_G:
            nc.tensor.wait_ge(s_bulk[1], 16)      # bulk2: group 1
        elif j == 2 * MM_PER_G:
            nc.tensor.wait_ge(s_bulk[2], 16)      # bulk3: groups 2-7 + rhs
        if j == N_PS - 1:
            nc.tensor.wait_ge(s_cpv, 1)       # slot 0 drained by half-copy a
        elif j == N_PS:
            nc.tensor.wait_ge(s_cpa, 1)       # slot 1 drained by half-copy b
        elif j > N_PS:
            # psum slot reuse: wait for the copy that drained it
            eng, pos = _copy_seq(j - N_PS)
            nc.tensor.wait_ge(s_cpv if eng == "v" else s_cpa, pos)
        nc.tensor.matmul(
            psum[(j + 1) % N_PS][:],
            wsb[:, _xt_col(j * 128):_xt_col(j * 128) + 128],
            wsb[:, _rhs_col(lb * PAIR * FC):_rhs_col(lb * PAIR * FC) + PAIR * FC],
            start=True, stop=True,
        ).then_inc(s_pe, 1)

    # ---- SP: output DMAs ----
    out_v4 = out[:].rearrange("(g j w) c -> g j w c", g=GROUPS, j=128, w=TPG)
    for g in range(GROUPS):
        if g == 0:
            # split DMAs: the stream starts right after matmul 0's first
            # half-copy
            w0 = 0
            for w1, nv, na in G0_SPLIT_W:
                if nv:
                    nc.sync.wait_ge(s_cpv, nv)
                if na:
                    nc.sync.wait_ge(s_cpa, na)
                nc.sync.dma_start(
                    out=out_v4[0][:, w0:w1, :],
                    in_=stage[0][:, w0 * FC:w1 * FC],
                ).then_inc(s_slot[0], 16)
                w0 = w1
            continue
        nc.sync.wait_ge(s_cpv, 4 * g + 4)
        nc.sync.wait_ge(s_cpa, 4 * g + 5)
        nc.sync.dma_start(out=out_v[g], in_=stage[g % N_STAGE][:]).then_inc(
            s_slot[g % N_STAGE], 16)

    ctx.close()
    nc.finalize()
    return nc


_NC_CACHE = None
_LAST_RESULTS = None  # BassKernelResults of the most recent run (for profiling)


def kernel(z, mask, initial_grid, W_pe, b_pe, W_clip, b_clip, emb_table,
           W_final, b_final):
    global _NC_CACHE, _LAST_RESULTS
    import ml_dtypes
    from concourse import bass_utils

    bf = ml_dtypes.bfloat16
    Q_all, r_all = _precompute(z, W_pe, b_pe, W_clip, b_clip, emb_table,
                               W_final, b_final)
    Qs = _split2(Q_all)                                 # 2 x [3, 180]
    X = np.ascontiguousarray(np.asarray(initial_grid), dtype=np.float32)
    g_cols = MM_PER_G * 128

    in_maps = []
    for c in range(NCORES):
        Xc = X[B_PER_CORE * c:B_PER_CORE * (c + 1)].reshape(PTS, NFEATS)
        # point p = g*2048 + j*16 + w lives at tile (g, w), psum partition j
        X4 = Xc.reshape(GROUPS, 128, TPG, NFEATS).transpose(3, 0, 2, 1)
        ch = _split2(X4)                                # 2 x [3, 8, 16, 128]
        A = np.empty((GROUPS, TPG, KR, 128), np.float32)
        for k in range(NFEATS):
            for m in range(3):
                A[:, :, 3 * k + m, :] = ch[XCH[m]][k]
        A[:, :, 9:11, :] = 1.0                          # bias rows
        # matmul s covers tiles (2*(s%8), 2*(s%8)+1) of group s//8;
        # stationary rows 11a.. hold tile a of the pair
        xt_host = (A.reshape(GROUPS, MM_PER_G, PAIR, KR, 128)
                   .transpose(2, 3, 0, 1, 4)
                   .reshape(K2, XT_COLS))

        rhs_host = np.zeros((K2, RHS_COLS), np.float32)
        for lb in range(B_PER_CORE):
            rs = _split2(r_all[B_PER_CORE * c + lb])    # 2 x [180]
            R = np.empty((KR, FC), np.float32)
            for k in range(NFEATS):
                for m in range(3):
                    R[3 * k + m] = Qs[QCH[m]][k]
            R[9:11] = np.stack(rs)
            for a in range(PAIR):                       # block-diagonal
                rhs_host[KR * a:KR * (a + 1),
                         lb * PAIR * FC + FC * a: lb * PAIR * FC + FC * (a + 1)] = R
        xt_host = xt_host.astype(bf)
        rhs_host = rhs_host.astype(bf)
        in_maps.append({
            "head": np.ascontiguousarray(
                np.concatenate([xt_host[:, :HEAD_X],
                                rhs_host[:, :PAIR * FC]], axis=1)),
            "bulk1": np.ascontiguousarray(xt_host[:, HEAD_X:g_cols]),
            "bulk2": np.ascontiguousarray(xt_host[:, g_cols:2 * g_cols]),
            "bulk3": np.ascontiguousarray(
                np.concatenate([xt_host[:, 2 * g_cols:],
                                rhs_host[:, PAIR * FC:]], axis=1)),
        })

    if _NC_CACHE is None:
        _NC_CACHE = _build_bass()
    res = bass_utils.run_bass_kernel_spmd(
        _NC_CACHE, in_maps, core_ids=list(range(NCORES))
    )
    _LAST_RESULTS = res

    out = np.empty((BS, NJOINTS, NFEATS, NFRAMES), np.float32)
    for c in range(NCORES):
        out[B_PER_CORE * c:B_PER_CORE * (c + 1)] = (
            res.results[c]["out"].reshape(B_PER_CORE, NJOINTS, NFEATS, NFRAMES)
        )
    return out


# revision 23
# speedup vs baseline: 1.1799x; 1.1799x over previous
"""Trainium2 Bass kernel for nn_Decoder_TRANSFORMER_14791867367496.

The reference decoder is affine in the positions: each frame step is
    pos_{t+1} = pos_t @ M + (d_t[b] + g[b,j]),   M = I + W_pe @ W3  (3x3)
(with W_final = [W1; W2; W3] split along its 768 input rows), so the whole
60-step scan has a closed form

    out[b, j, :, t] = X[b, j, :] @ Q_t + r_t[b, :]

where X = initial_grid,
    Q_t = M^t + (W_pe @ W2) @ S_t,          S_t = sum_{k<t} M^k
    r_t[b] = h @ S_t + D_t[b],              D_t = sum_{s=1..t} d_s M^{t-s}
    d_t[b] = (emb_table[t] + z @ W_clip + b_clip) @ W1
    h      = b_pe @ (W2 + W3) + b_final

All of Q/r are tiny (3x3 / per-batch 3-vectors) and are computed on the host
in float64.  The device kernel is then a single affine map per point
([3 feats + bias] -> 180 outputs) and is purely output-bandwidth bound
(94 MB of f32 output; measured per-core DMA saturation ~422 GB/s).

Precision trick: fp32 operands are split into two bf16 chunks
(x = x0+x1, 8 mantissa bits each) and the cross terms with a+b <= 1 are
summed IN A SINGLE MATMUL by stacking them along the contraction dim:
rows [x0 x0 x1] paired against [q0 q1 q0] per feature, plus two ones-rows
paired against the two bf16 chunks of the per-batch bias r.  bf16 products
are exact in fp32; dropped second-order terms are ~2^-18 (measured rel err
2.4e-6 vs the 2e-4 gate).  Per point-pair-tile the K-stack is 11 rows x 2
tiles = K=22, N=2*180=360 (block-diagonal rhs).

Sharding: data-parallel over batch - each of the 8 cores handles 4 batches
(16384 points = 128 point-tiles = 64 packed matmuls).  Output streams out
in fully-linear ~1.47 MB DMAs, the first group split 1/1/2/2/2 matmuls so
the stream starts right after matmul 0's copy.

Ramp/pipeline design (steady state is DMA-saturated, so exec time =
stream time + ramp latency; DMA completion semaphores lag the trigger's
descriptor-gen by a fixed ~1.4 us, descriptor-gen blocks the issuing
engine ~0.6-1.0 us per DMA, and all SBUF operands live in ONE sbuf tensor
so multi-part input loads stay fully linear):
  - SP ring carries ONE tiny head DMA (matmul 0's stationary + rhs
    columns) and then only the output DMAs.
  - ACT ring carries the bulk input as three linear DMAs (rest of group
    0; group 1; groups 2-7 + remaining rhs), each with its own completion
    semaphore (a DMA's +16 arrives as 16 per-engine +1 increments that
    interleave across DMAs, so cumulative thresholds on a shared
    semaphore would be racy).  The ACT engine is free again by the time
    its first PSUM copy is due.
  - Six stage buffers so the PSUM->SBUF copies never wait on the ~1.4 us
    out-DMA completion latency chain.
PE runs the matmuls, DVE/ACT alternate PSUM->SBUF copies, SP streams the
output.  The device program is raw Bacc with hand-rolled per-edge
semaphores.
"""

import numpy as np

BS, NFRAMES, NJOINTS, NFEATS, LATENT, CLIP = 32, 60, 4096, 3, 256, 512
NCORES = 8
B_PER_CORE = BS // NCORES                  # 4
PTS = B_PER_CORE * NJOINTS                 # 16384 points per core
NTILES = PTS // 128                        # 128 point-tiles per core
GROUPS = 8                                 # output DMA groups
TPG = NTILES // GROUPS                     # 16 tiles per group
FC = NFEATS * NFRAMES                      # 180 output columns per point
KR = 11                                    # K-stack rows per tile (3*3 + 2 bias)
PAIR = 2                                   # tiles fused per matmul
MM_PER_G = TPG // PAIR                     # 8 matmuls per group
NMM = GROUPS * MM_PER_G                    # 64 matmuls per core
XCH = [0, 0, 1]                            # x-chunk index per K row (per feat)
QCH = [0, 1, 0]                            # q-chunk index per K row (per feat)
K2 = PAIR * KR                             # matmul contraction dim (22)
XT_COLS = NMM * 128                        # 8192 stationary columns
RHS_COLS = B_PER_CORE * PAIR * FC          # 1440 moving columns
HEAD_X = 2 * 128                           # stationary cols in the head DMA
HEAD = HEAD_X + PAIR * FC                  # head DMA cols (616)
W_COLS = XT_COLS + RHS_COLS                # single sbuf tensor width (9632)
# group-0 output split in point-tiles per DMA; matmul 0 is itself split
# into two single-tile matmuls (half-N) with their own half-copies so the
# first out-DMA fires as early as possible.
# SP-ring share of group 0's splits, (w0, w1, cpv_need, cpa_need); tiles
# 1:2 and 4:8 go out on the ACT ring in-stream with its copies
G0_SPLIT_W = ((0, 1, 1, 0), (2, 4, 0, 2), (8, 16, 4, 5))
G0_NDMA = 5                                # total group-0 out-DMAs


def _xt_col(c):
    """wsb column of stationary (xt) column c."""
    return c if c < HEAD_X else PAIR * FC + c


def _rhs_col(r):
    """wsb column of moving (rhs) column r."""
    return HEAD_X + r if r < PAIR * FC else XT_COLS + r


def _split2(a):
    """Split f32 array into two bf16 chunks whose sum reproduces ~16
    mantissa bits.  Returned as f32 arrays holding bf16-representable
    values."""
    import ml_dtypes
    bf = ml_dtypes.bfloat16
    a = np.asarray(a, np.float32)
    a0 = a.astype(bf).astype(np.float32)
    a1 = (a - a0).astype(bf).astype(np.float32)
    return a0, a1


def _precompute(z, W_pe, b_pe, W_clip, b_clip, emb_table, W_final, b_final):
    """Host-side f64 computation of the closed-form coefficients.

    Returns Q_all [3, 180] and r_all [32, 180], column layout c = f*60 + t
    (matching the [.., 3, 60] innermost layout of the output)."""
    f64 = np.float64
    W_pe64 = np.asarray(W_pe, f64)
    W_fin = np.asarray(W_final, f64)
    W1, W2, W3 = W_fin[:LATENT], W_fin[LATENT:2 * LATENT], W_fin[2 * LATENT:]
    M = np.eye(3) + W_pe64 @ W3
    Gm = W_pe64 @ W2
    b_pe64 = np.asarray(b_pe, f64)
    h = b_pe64 @ W2 + b_pe64 @ W3 + np.asarray(b_final, f64)
    z_proj = np.asarray(z, f64) @ np.asarray(W_clip, f64) + np.asarray(b_clip, f64)
    d = (np.asarray(emb_table, f64)[None, :, :] + z_proj[:, None, :]) @ W1  # [32,60,3]

    Q = np.zeros((NFRAMES, 3, 3))
    R = np.zeros((NFRAMES, BS, 3))
    Q[0] = np.eye(3)
    Mt = np.eye(3)
    S = np.zeros((3, 3))
    D = np.zeros((BS, 3))
    for t in range(1, NFRAMES):
        S = S + Mt
        Mt = Mt @ M
        D = D @ M + d[:, t, :]
        Q[t] = Mt + Gm @ S
        R[t] = h @ S + D
    Q_all = Q.transpose(1, 2, 0).reshape(3, FC)     # [k, f*60+t]
    r_all = R.transpose(1, 2, 0).reshape(BS, FC)    # [b, f*60+t]
    return Q_all.astype(np.float32), r_all.astype(np.float32)


N_PS = 8      # psum slots (one bank each; a group cycles all 8)
N_STAGE = 6   # stage buffers


def _copy_seq(j):
    """(engine, 1-based position of copy j within that engine's stream).

    Copies alternate DVE/ACT by matmul index so both engines share every
    group's copy wall.  Matmul 0's copy is split into two halves (DVE pos
    1 / ACT pos 1), which shifts the ACT positions by one."""
    if j % 2 == 0:
        return "v", j // 2 + 1
    return "a", (j + 1) // 2 + 1


def _build_bass():
    import concourse.mybir as mybir
    from concourse import bacc
    from concourse.bass import ts

    f32 = mybir.dt.float32
    bf16 = mybir.dt.bfloat16
    nc = bacc.Bacc(None, target_bir_lowering=False)
    # bulk boundaries in xt-column space: rest of group 0 | group 1 |
    # groups 2-7 (the last bulk also carries the remaining rhs columns)
    b1_cols = MM_PER_G * 128 - HEAD_X
    b2_cols = MM_PER_G * 128
    b3_cols = (GROUPS - 2) * MM_PER_G * 128 + (B_PER_CORE - 1) * PAIR * FC
    head = nc.dram_tensor("head", [K2, HEAD], bf16, kind="ExternalInput")
    bulk1 = nc.dram_tensor("bulk1", [K2, b1_cols], bf16, kind="ExternalInput")
    bulk2 = nc.dram_tensor("bulk2", [K2, b2_cols], bf16, kind="ExternalInput")
    bulk3 = nc.dram_tensor("bulk3", [K2, b3_cols], bf16, kind="ExternalInput")
    out = nc.dram_tensor("out", [PTS, FC], f32, kind="ExternalOutput")
    out_v = out[:].rearrange("(g j w) c -> g j (w c)", g=GROUPS, j=128, w=TPG)

    from contextlib import ExitStack
    ctx = ExitStack()
    wsb = ctx.enter_context(nc.sbuf_tensor("wsb", [K2, W_COLS], bf16))
    stage = [ctx.enter_context(
        nc.sbuf_tensor(f"stage{i}", [128, TPG * FC], f32))
        for i in range(N_STAGE)]
    psum = [ctx.enter_context(
        nc.psum_tensor(f"psum{i}", [128, PAIR * FC], f32))
        for i in range(N_PS)]
    s_head = ctx.enter_context(nc.semaphore("s_head"))
    # one semaphore per bulk DMA: a DMA's +16 arrives as 16 per-engine +1
    # increments which interleave across DMAs on the same queue, so
    # cumulative thresholds on a shared semaphore would be racy
    s_bulk = [ctx.enter_context(nc.semaphore(f"s_bulk{i}")) for i in range(3)]
    s_pe = ctx.enter_context(nc.semaphore("s_pe"))
    s_cpv = ctx.enter_context(nc.semaphore("s_cpv"))
    s_cpa = ctx.enter_context(nc.semaphore("s_cpa"))
    s_slot = [ctx.enter_context(nc.semaphore(f"s_slot{i}"))
              for i in range(N_STAGE)]

    # ---- input DMAs ----
    # SP ring: one tiny head DMA (matmul 0+1 stationary cols + local batch
    # 0's rhs), then nothing but output.
    nc.sync.dma_start(out=wsb[:, :HEAD], in_=head[:]).then_inc(s_head, 16)
    # ACT ring: bulk input
    c0 = HEAD
    for i, (t, w) in enumerate(((bulk1, b1_cols), (bulk2, b2_cols),
                                (bulk3, b3_cols))):
        nc.scalar.dma_start(out=wsb[:, c0:c0 + w],
                            in_=t[:]).then_inc(s_bulk[i], 16)
        c0 += w

    out_v4 = out[:].rearrange("(g j w) c -> g j w c", g=GROUPS, j=128, w=TPG)

    def g0_out(engine, w0, w1):
        """Group-0 ramp out-DMA for tiles [w0, w1) on the given ring."""
        return engine.dma_start(
            out=out_v4[0][:, w0:w1, :],
            in_=stage[0][:, w0 * FC:w1 * FC],
        ).then_inc(s_slot[0], 16)

    def copies(engine, s_cp_self, g, parity):
        st = stage[g % N_STAGE]
        for sp in range(parity, MM_PER_G, 2):
            j = g * MM_PER_G + sp
            if sp == parity and g >= N_STAGE:
                # stage slot reuse: wait for every out-DMA that read it
                engine.wait_ge(s_slot[g % N_STAGE],
                               16 * slot_reads_before[g])
            if g == 0 and sp == parity:
                # group 0 starts with the two half-copies of matmul 0
                # (psum banks 0 and 1 hold the two half-N results)
                engine.wait_ge(s_pe, parity + 1)
                half = psum[parity][:, :FC]
                dst = st[:, parity * FC:(parity + 1) * FC]
                if parity == 0:
                    nc.vector.tensor_copy(out=dst, in_=half).then_inc(
                        s_cp_self, 1)
                    continue
                nc.scalar.copy(out=dst, in_=half).then_inc(s_cp_self, 1)
                # tile 1 goes out on the ACT ring, concurrently with the
                # SP ring's tile-0 DMA: the ramp is descriptor-dispatch
                # limited, so two rings double the early dispatch rate.
                # copy0b precedes in-stream, so no semaphore is needed.
                g0_out(nc.scalar, 1, 2)
            engine.wait_ge(s_pe, j + 2)   # +1 for the extra half matmul
            if parity == 0:
                nc.vector.tensor_copy(
                    out=st[:, ts(sp, PAIR * FC)],
                    in_=psum[(j + 1) % N_PS][:],
                ).then_inc(s_cp_self, 1)
            else:
                nc.scalar.copy(
                    out=st[:, ts(sp, PAIR * FC)],
                    in_=psum[(j + 1) % N_PS][:],
                ).then_inc(s_cp_self, 1)
                if g == 0 and sp == 3:
                    # tiles 4:8 out on the ACT ring (needs copy2 from DVE;
                    # own copy3 precedes in-stream)
                    nc.scalar.wait_ge(s_cpv, 2)
                    g0_out(nc.scalar, 4, 8)

    # number of completed out-DMA incs required on slot g%N_STAGE before
    # group g may overwrite it (group 0's stage goes out as split DMAs)
    dma_count = {0: G0_NDMA}
    slot_reads_before = {}
    seen = [0] * N_STAGE
    for g in range(GROUPS):
        slot_reads_before[g] = seen[g % N_STAGE]
        seen[g % N_STAGE] += dma_count.get(g, 1)

    for g in range(GROUPS):
        copies(nc.scalar, s_cpa, g, 1)

    # ---- DVE: even-slot copies ----
    for g in range(GROUPS):
        copies(nc.vector, s_cpv, g, 0)

    # ---- PE: matmuls ----
    # matmul 0 runs as two single-tile (half-N) matmuls so its first half
    # copy (and the first out-DMA) fires half a matmul earlier
    for j in range(NMM):
        g, sp = j // MM_PER_G, j % MM_PER_G
        lb = g // 2
        if j == 0:
            nc.tensor.wait_ge(s_head, 16)
            # half-N matmuls with the full K=22 stationary: the rhs is
            # block-diagonal, so the off-tile stationary rows multiply
            # zero columns and the halves stay exact.  Each half gets its
            # own psum bank (PE writes must start at a bank base), which
            # shifts all later slot assignments by one.
            r0 = _rhs_col(0)
            for a in range(PAIR):
                nc.tensor.matmul(
                    psum[a][:, :FC],
                    wsb[:, 0:128],
                    wsb[:, r0 + a * FC:r0 + (a + 1) * FC],
                    start=True, stop=True,
                ).then_inc(s_pe, 1)
            continue
        if j == 2:
            nc.tensor.wait_ge(s_bulk[0], 16)      # bulk1: rest of group 0
        elif j == MM_PER_G:
            nc.tensor.wait_ge(s_bulk[1], 16)      # bulk2: group 1
        elif j == 2 * MM_PER_G:
            nc.tensor.wait_ge(s_bulk[2], 16)      # bulk3: groups 2-7 + rhs
        if j == N_PS - 1:
            nc.tensor.wait_ge(s_cpv, 1)       # slot 0 drained by half-copy a
        elif j == N_PS:
            nc.tensor.wait_ge(s_cpa, 1)       # slot 1 drained by half-copy b
        elif j > N_PS:
            # psum slot reuse: wait for the copy that drained it
            eng, pos = _copy_seq(j - N_PS)
            nc.tensor.wait_ge(s_cpv if eng == "v" else s_cpa, pos)
        nc.tensor.matmul(
            psum[(j + 1) % N_PS][:],
            wsb[:, _xt_col(j * 128):_xt_col(j * 128) + 128],
            wsb[:, _rhs_col(lb * PAIR * FC):_rhs_col(lb * PAIR * FC) + PAIR * FC],
            start=True, stop=True,
        ).then_inc(s_pe, 1)

    # ---- SP: output DMAs ----
    # (tiles 1:2 and 4:8 of group 0 go out on the ACT ring, interleaved
    # with the copies above, so both rings dispatch during the ramp)
    for g in range(GROUPS):
        if g == 0:
            # split DMAs: the stream starts right after matmul 0's first
            # half-copy
            for w0, w1, nv, na in G0_SPLIT_W:
                if nv:
                    nc.sync.wait_ge(s_cpv, nv)
                if na:
                    nc.sync.wait_ge(s_cpa, na)
                g0_out(nc.sync, w0, w1)
            continue
        nc.sync.wait_ge(s_cpv, 4 * g + 4)
        nc.sync.wait_ge(s_cpa, 4 * g + 5)
        nc.sync.dma_start(out=out_v[g], in_=stage[g % N_STAGE][:]).then_inc(
            s_slot[g % N_STAGE], 16)

    ctx.close()
    nc.finalize()
    return nc


_NC_CACHE = None
_LAST_RESULTS = None  # BassKernelResults of the most recent run (for profiling)


def kernel(z, mask, initial_grid, W_pe, b_pe, W_clip, b_clip, emb_table,
           W_final, b_final):
    global _NC_CACHE, _LAST_RESULTS
    import ml_dtypes
    from concourse import bass_utils

    bf = ml_dtypes.bfloat16
    Q_all, r_all = _precompute(z, W_pe, b_pe, W_clip, b_clip, emb_table,
                               W_final, b_final)
    Qs = _split2(Q_all)                                 # 2 x [3, 180]
    X = np.ascontiguousarray(np.asarray(initial_grid), dtype=np.float32)
    g_cols = MM_PER_G * 128

    in_maps = []
    for c in range(NCORES):
        Xc = X[B_PER_CORE * c:B_PER_CORE * (c + 1)].reshape(PTS, NFEATS)
        # point p = g*2048 + j*16 + w lives at tile (g, w), psum partition j
        X4 = Xc.reshape(GROUPS, 128, TPG, NFEATS).transpose(3, 0, 2, 1)
        ch = _split2(X4)                                # 2 x [3, 8, 16, 128]
        A = np.empty((GROUPS, TPG, KR, 128), np.float32)
        for k in range(NFEATS):
            for m in range(3):
                A[:, :, 3 * k + m, :] = ch[XCH[m]][k]
        A[:, :, 9:11, :] = 1.0                          # bias rows
        # matmul s covers tiles (2*(s%8), 2*(s%8)+1) of group s//8;
        # stationary rows 11a.. hold tile a of the pair
        xt_host = (A.reshape(GROUPS, MM_PER_G, PAIR, KR, 128)
                   .transpose(2, 3, 0, 1, 4)
                   .reshape(K2, XT_COLS))

        rhs_host = np.zeros((K2, RHS_COLS), np.float32)
        for lb in range(B_PER_CORE):
            rs = _split2(r_all[B_PER_CORE * c + lb])    # 2 x [180]
            R = np.empty((KR, FC), np.float32)
            for k in range(NFEATS):
                for m in range(3):
                    R[3 * k + m] = Qs[QCH[m]][k]
            R[9:11] = np.stack(rs)
            for a in range(PAIR):                       # block-diagonal
                rhs_host[KR * a:KR * (a + 1),
                         lb * PAIR * FC + FC * a: lb * PAIR * FC + FC * (a + 1)] = R
        xt_host = xt_host.astype(bf)
        rhs_host = rhs_host.astype(bf)
        in_maps.append({
            "head": np.ascontiguousarray(
                np.concatenate([xt_host[:, :HEAD_X],
                                rhs_host[:, :PAIR * FC]], axis=1)),
            "bulk1": np.ascontiguousarray(xt_host[:, HEAD_X:g_cols]),
            "bulk2": np.ascontiguousarray(xt_host[:, g_cols:2 * g_cols]),
            "bulk3": np.ascontiguousarray(
                np.concatenate([xt_host[:, 2 * g_cols:],
                                rhs_host[:, PAIR * FC:]], axis=1)),
        })

    if _NC_CACHE is None:
        _NC_CACHE = _build_bass()
    res = bass_utils.run_bass_kernel_spmd(
        _NC_CACHE, in_maps, core_ids=list(range(NCORES))
    )
    _LAST_RESULTS = res

    out = np.empty((BS, NJOINTS, NFEATS, NFRAMES), np.float32)
    for c in range(NCORES):
        out[B_PER_CORE * c:B_PER_CORE * (c + 1)] = (
            res.results[c]["out"].reshape(B_PER_CORE, NJOINTS, NFEATS, NFRAMES)
        )
    return out
